# revision 18
# baseline (speedup 1.0000x reference)
"""Sliding-window GQA attention on 8 TRN2 NeuronCores.

Sharding: core c handles batch b=c//4 and kv-head pair 2*(c%4)..+1
(-> 4 query heads, 2 kv heads, all 2048 tokens of one batch).
Each core computes its heads' partial o-projection [2048, 3584];
the host sums the 4 partials per batch. No on-device collectives.

All heavy matmuls run in bf16 (fp32 PSUM accumulate). Host pre-lays-out
x^T / weights in [128-partition, ...] tiles so every DMA is contiguous.
RoPE cos/sin tables and the two 128x128 additive mask tiles (causal
diag, window edge) come from host; the QK-norm scale is applied
post-transpose where the head dim sits on partitions.
"""

import os
import numpy as np
import ml_dtypes

B, T, D, H = 2, 2048, 3584, 256
QH, KVH = 4, 2          # per-core q heads / kv heads
DC = D // 128           # 28 contract chunks
TBN = T // 128          # 16 token blocks
HC = H // 128           # 2 head-dim chunks
OC = QH * H // 128      # 8 out-proj contract chunks
SCALE = 0.0625
EPS = 1e-6
ROPE_BASE = 10000.0
WB = 1024 // 128        # window in blocks (8)
NEG = -1.0e30
NDOUT = D // 512        # 7 o-proj column chunks

BF16 = ml_dtypes.bfloat16

_cached = {}


def _build():
    import concourse.bass as bass
    import concourse.mybir as mybir
    import concourse.tile as tile
    from concourse import bacc
    from concourse.masks import make_identity

    f32 = mybir.dt.float32
    bf16 = mybir.dt.bfloat16
    AF = mybir.ActivationFunctionType

    nc = bacc.Bacc(None, target_bir_lowering=False)

    xT_d = nc.dram_tensor("xT", [128, DC, T], bf16, kind="ExternalInput")
    wq_d = nc.dram_tensor("wq", [128, DC, QH * H], bf16, kind="ExternalInput")
    wkv_d = nc.dram_tensor("wkv", [128, DC, 2 * KVH * H], bf16, kind="ExternalInput")
    wo_d = nc.dram_tensor("wo", [128, OC, D], bf16, kind="ExternalInput")
    cos_d = nc.dram_tensor("cos", [128, TBN, 128], f32, kind="ExternalInput")
    sin_d = nc.dram_tensor("sin", [128, TBN, 128], f32, kind="ExternalInput")
    qsc_d = nc.dram_tensor("qsc", [128, HC], f32, kind="ExternalInput")
    ksc_d = nc.dram_tensor("ksc", [128, HC], f32, kind="ExternalInput")
    mdiag_d = nc.dram_tensor("mdiag", [128, 128], f32, kind="ExternalInput")
    medge_d = nc.dram_tensor("medge", [128, 128], f32, kind="ExternalInput")
    out_d = nc.dram_tensor("out", [T, D], f32, kind="ExternalOutput")

    DCQ = DC // 4  # 7 d-chunks per weight quarter

    with tile.TileContext(nc) as tc:
        with (
            tc.tile_pool(name="persist", bufs=1) as pers,
            tc.tile_pool(name="wpool", bufs=6) as wpool,
        ):
            qT = pers.tile([128, HC, QH, T], bf16)     # q^T  [h, hc, head, t]
            kT = pers.tile([128, HC, KVH, T], bf16)    # k^T  [h, hc, kv, s]
            vS = pers.tile([128, TBN, KVH, H], bf16)   # v    [s, sblock, kv, h]
            ident = pers.tile([128, 128], bf16)
            make_identity(nc, ident)
            epsb = pers.tile([128, 1], f32)
            nc.gpsimd.memset(epsb, EPS)
            epsb2 = pers.tile([128, 1], f32)
            nc.gpsimd.memset(epsb2, EPS / (SCALE * SCALE))
            rqs = pers.tile([128, TBN, QH], f32)   # SCALE/rms(q) per (t, head)

            def load_w_quarters(dram, rows, cols, eng=None):
                parts = []
                for qi in range(4):
                    wt = wpool.tile([128, rows // 4, cols], bf16, tag="w",
                                    name=f"w_{qi}")
                    (eng or nc.sync).dma_start(
                        wt, dram[:, qi * (rows // 4):(qi + 1) * (rows // 4), :])
                    parts.append(wt)
                return parts

            # ---------------- phase 1: projections ----------------
            with (
                tc.tile_pool(name="tabs", bufs=1) as ptab,
                tc.tile_pool(name="xt", bufs=3) as pxt,
                tc.tile_pool(name="scr", bufs=3) as scr,
                tc.tile_pool(name="ppq", bufs=6, space=bass.MemorySpace.PSUM) as ppq,
                tc.tile_pool(name="ptr", bufs=2, space=bass.MemorySpace.PSUM) as ptrp,
            ):
                # wq quarter 0 gets the sync queue to itself; quarters 1-3
                # are issued on the paced gpsimd queue between early blocks.
                WSPLIT = (4, 8, 8, 8)
                WOFF = (0, 4, 12, 20)
                wq_p = [wpool.tile([128, WSPLIT[qi], QH * H], bf16, tag="w",
                                   name=f"wq_{qi}") for qi in range(4)]
                nc.sync.dma_start(wq_p[0], wq_d[:, :4, :])

                xts0 = []
                for tbe in range(3):
                    xte = pxt.tile([128, DC, 128], bf16, tag="xt", name="xt0")
                    nc.gpsimd.dma_start(xte, xT_d[:, :, tbe * 128:(tbe + 1) * 128])
                    xts0.append(xte)
                for qi in range(1, 4):
                    nc.gpsimd.dma_start(
                        wq_p[qi], wq_d[:, WOFF[qi]:WOFF[qi] + WSPLIT[qi], :])

                cost = ptab.tile([128, TBN, 128], f32)
                sint = ptab.tile([128, TBN, 128], f32)
                qsc = ptab.tile([128, HC], f32)
                ksc = ptab.tile([128, HC], f32)
                nc.gpsimd.dma_start(cost, cos_d[:])
                nc.gpsimd.dma_start(sint, sin_d[:])
                nc.gpsimd.dma_start(qsc, qsc_d[:])
                nc.gpsimd.dma_start(ksc, ksc_d[:])

                def proj_epilogue(pq, j, tb, scv, dstT, slot, defer_rstd=False):
                    """norm+rope head j of psum pq -> transpose into dstT[:, hc, slot, tb].
                    The QK-norm per-h scale is applied post-transpose (h on
                    partitions). For q (defer_rstd), 1/rms is not applied here;
                    SCALE/rms(q) is stored in rqs and folded into the exp scale."""
                    sq = scr.tile([128, H], f32, tag="sq")
                    ssq = scr.tile([128, 1], f32, tag="ssq")
                    nc.scalar.activation(sq, pq[:, j, :], AF.Square, accum_out=ssq)
                    std = scr.tile([128, 1], f32, tag="std")
                    if defer_rstd:
                        nc.scalar.activation(std, ssq, AF.Sqrt, bias=epsb2[:, 0:1],
                                             scale=1.0 / (H * SCALE * SCALE))
                        nc.vector.reciprocal(rqs[:, tb, slot, None], std)
                    else:
                        nc.scalar.activation(std, ssq, AF.Sqrt, bias=epsb[:, 0:1],
                                             scale=1.0 / H)
                        rstd = scr.tile([128, 1], f32, tag="rstd")
                        nc.vector.reciprocal(rstd, std)
                        rb = rstd[:, 0:1].to_broadcast((128, 128))
                    x1 = pq[:, j, 0:128]
                    x2 = pq[:, j, 128:256]
                    t1 = scr.tile([128, 128], f32, tag="t1")
                    qr = scr.tile([128, H], bf16, tag="qr")
                    t2 = scr.tile([128, 128], f32, tag="t2")
                    nc.vector.tensor_mul(t1, x1, cost[:, tb, :])
                    nc.vector.tensor_mul(t2, x2, sint[:, tb, :])
                    if defer_rstd:
                        nc.vector.tensor_sub(qr[:, 0:128], t1, t2)
                    else:
                        nc.vector.tensor_sub(t1, t1, t2)
                        nc.vector.tensor_mul(qr[:, 0:128], t1, rb)
                    nc.vector.tensor_mul(t1, x2, cost[:, tb, :])
                    nc.vector.tensor_mul(t2, x1, sint[:, tb, :])
                    if defer_rstd:
                        nc.vector.tensor_add(qr[:, 128:256], t1, t2)
                    else:
                        nc.vector.tensor_add(t1, t1, t2)
                        nc.vector.tensor_mul(qr[:, 128:256], t1, rb)
                    for hc in range(HC):
                        ptr = ptrp.tile([128, 128], bf16, tag="ptr")
                        nc.tensor.transpose(ptr, qr[:, hc * 128:(hc + 1) * 128], ident)
                        nc.scalar.activation(
                            dstT[:, hc, slot, tb * 128:(tb + 1) * 128], ptr,
                            AF.Copy, scale=scv[:, hc:hc + 1])

                def proj_block(tb, w_p, nhead, xt=None, woff=None):
                    if xt is None:
                        xt = pxt.tile([128, DC, 128], bf16, tag="xt")
                        nc.gpsimd.dma_start(xt, xT_d[:, :, tb * 128:(tb + 1) * 128])
                    def wsel(dc):
                        if woff is None:
                            return w_p[dc // DCQ], dc % DCQ
                        for qi in range(3, -1, -1):
                            if dc >= woff[qi]:
                                return w_p[qi], dc - woff[qi]
                    pqa = ppq.tile([128, 2, H], f32, tag="pq", name="pqa")
                    pqb = ppq.tile([128, 2, H], f32, tag="pq", name="pqb")
                    for dc in range(DC):
                        lhsT = xt[:, dc, :]
                        wt, dcl = wsel(dc)
                        nc.tensor.matmul(pqa[:, :, :], lhsT, wt[:, dcl, 0:512],
                                         start=(dc == 0), stop=(dc == DC - 1))
                        nc.tensor.matmul(pqb[:, :, :], lhsT, wt[:, dcl, 512:1024],
                                         start=(dc == 0), stop=(dc == DC - 1))
                    return (pqa, pqb)

                # --- 1a: Q ---
                wkv_p = None
                for tb in range(TBN):
                    pq2 = proj_block(tb, wq_p, QH,
                                     xt=xts0[tb] if tb < 3 else None,
                                     woff=WOFF)
                    for j in range(QH):
                        proj_epilogue(pq2[j // 2], j % 2, tb, qsc, qT, j,
                                      defer_rstd=True)
                    if tb == 7:
                        # paced prefetch: issued on the xt DMA queue mid-phase
                        wkv_p = load_w_quarters(wkv_d, DC, 2 * KVH * H,
                                                eng=nc.gpsimd)

                # --- 1b: K and V ---
                for tb in range(TBN):
                    pq2 = proj_block(tb, wkv_p, 2 * KVH)
                    for kv in range(KVH):
                        proj_epilogue(pq2[0], kv, tb, ksc, kT, kv)
                    for kv in range(KVH):
                        nc.vector.tensor_copy(vS[:, tb, kv, :], pq2[1][:, kv, :])
                    if tb == 7:
                        wo_p = []
                        for qi in range(4):
                            wt = wpool.tile([128, OC // 4, D], bf16, tag="w",
                                            name=f"wo_{qi}")
                            nc.gpsimd.dma_start(
                                wt, wo_d[:, qi * (OC // 4):(qi + 1) * (OC // 4), :])
                            wo_p.append(wt)

            # ---------------- phase 2: attention + o-proj ----------------
            with (
                tc.tile_pool(name="att", bufs=2) as att,
                tc.tile_pool(name="ysb", bufs=1) as pys,
                tc.tile_pool(name="pl", bufs=2, space=bass.MemorySpace.PSUM) as plp,
                tc.tile_pool(name="pt2", bufs=3, space=bass.MemorySpace.PSUM) as pt2,
                tc.tile_pool(name="po", bufs=1, space=bass.MemorySpace.PSUM) as pop,
                tc.tile_pool(name="py", bufs=2, space=bass.MemorySpace.PSUM) as pyp,
            ):
                mdiag = att.tile([128, 128], f32, tag="mdiag", bufs=1)
                medge = att.tile([128, 128], f32, tag="medge", bufs=1)
                nc.sync.dma_start(mdiag, mdiag_d[:])
                nc.sync.dma_start(medge, medge_d[:])


                for tb in range(TBN):
                    sb0 = max(0, tb - WB)
                    ns = tb - sb0 + 1
                    sw = ns * 128
                    dg = (tb - sb0) * 128  # diag block offset within row
                    outsb = att.tile([128, QH, H], bf16, tag="outsb")
                    for j in range(QH):
                        kv = j // 2
                        # QK in 512-wide chunks; per-chunk exp so PE/ACT pipeline
                        pcs = []
                        sumes = []
                        for c0 in range(0, sw, 512):
                            cols = min(512, sw - c0)
                            pl = plp.tile([128, 512], f32, tag="pl", name="pl")
                            for hc in range(HC):
                                nc.tensor.matmul(
                                    pl[:, :cols],
                                    qT[:, hc, j, tb * 128:(tb + 1) * 128],
                                    kT[:, hc, kv,
                                       sb0 * 128 + c0: sb0 * 128 + c0 + cols],
                                    start=(hc == 0), stop=(hc == HC - 1))
                            if tb >= WB and c0 == 0:
                                nc.vector.tensor_add(pl[:, 0:128], pl[:, 0:128], medge)
                            if c0 <= dg < c0 + cols:
                                off = dg - c0
                                nc.vector.tensor_add(pl[:, off:off + 128],
                                                     pl[:, off:off + 128], mdiag)
                            pc = att.tile([128, 512], bf16, tag="probs", bufs=6,
                                          name="pc")
                            se = att.tile([128, 1], f32, tag="sume", bufs=8, name="se")
                            nc.scalar.activation(pc[:, :cols], pl[:, :cols], AF.Exp,
                                                 scale=rqs[:, tb, j, None],
                                                 accum_out=se)
                            pcs.append(pc)
                            sumes.append(se)
                        tot = sumes[0]
                        for se in sumes[1:]:
                            t2 = att.tile([128, 1], f32, tag="sume", bufs=8, name="se2")
                            nc.vector.tensor_add(t2, tot, se)
                            tot = t2
                        recip = att.tile([128, 1], f32, tag="recip")
                        nc.vector.reciprocal(recip, tot)
                        pT = att.tile([128, 9, 128], bf16, tag="pT")
                        for s in range(ns):
                            ptr = pt2.tile([128, 128], bf16, tag="ptr2")
                            nc.tensor.transpose(
                                ptr, pcs[s // 4][:, (s % 4) * 128:(s % 4 + 1) * 128],
                                ident)
                            if s % 3 != 2:
                                nc.vector.tensor_copy(pT[:, s, :], ptr)
                            else:
                                nc.scalar.activation(pT[:, s, :], ptr, AF.Copy)
                        po = pop.tile([128, H], f32, tag="po")
                        for s in range(ns):
                            nc.tensor.matmul(po, pT[:, s, :], vS[:, sb0 + s, kv, :],
                                             start=(s == 0), stop=(s == ns - 1))
                        nc.vector.tensor_mul(outsb[:, j, :], po,
                                             recip[:, 0:1].to_broadcast((128, H)))
                    outT = att.tile([128, OC, 128], bf16, tag="outT")
                    for c in range(OC):
                        ptr = pt2.tile([128, 128], bf16, tag="ptr2")
                        nc.tensor.transpose(
                            ptr, outsb[:, c // 2, (c % 2) * 128:(c % 2 + 1) * 128],
                            ident)
                        if c % 2 == 0:
                            nc.vector.tensor_copy(outT[:, c, :], ptr)
                        else:
                            nc.scalar.activation(outT[:, c, :], ptr, AF.Copy)
                    ysb = pys.tile([128, D], f32, tag="y")
                    for dx in range(NDOUT):
                        py = pyp.tile([128, 512], f32, tag="py")
                        for c in range(OC):
                            nc.tensor.matmul(py, outT[:, c, :],
                                             wo_p[c // 2][:, c % 2,
                                                          dx * 512:(dx + 1) * 512],
                                             start=(c == 0), stop=(c == OC - 1))
                        if dx % 2 == 0:
                            nc.vector.tensor_copy(ysb[:, dx * 512:(dx + 1) * 512], py)
                        else:
                            nc.scalar.activation(ysb[:, dx * 512:(dx + 1) * 512], py,
                                                 AF.Copy)
                    nc.sync.dma_start(out_d[tb * 128:(tb + 1) * 128, :D // 2],
                                      ysb[:, :D // 2])
                    nc.sync.dma_start(out_d[tb * 128:(tb + 1) * 128, D // 2:],
                                      ysb[:, D // 2:])

    nc.compile()
    return nc


def _tile128(a):
    """[128*n, m] -> [128, n, m] with row index = chunk*128 + partition."""
    n = a.shape[0] // 128
    return np.ascontiguousarray(
        a.reshape(n, 128, *a.shape[1:]).transpose(1, 0, *range(2, a.ndim + 1)))


def _rope_tabs():
    j = np.arange(128, dtype=np.float64)
    ts = ROPE_BASE ** (2.0 * j / H)
    ang = np.arange(T, dtype=np.float64)[:, None] / ts[None, :]
    return (_tile128(np.cos(ang).astype(np.float32)),
            _tile128(np.sin(ang).astype(np.float32)))


def kernel(x, w_q, w_kv, w_o, q_norm_scale, k_norm_scale):
    from concourse.bass_utils import run_bass_kernel_spmd

    if "nc" not in _cached:
        _cached["nc"] = _build()
    nc = _cached["nc"]

    x = np.asarray(x, np.float32)
    w_q = np.asarray(w_q, np.float32)
    w_kv = np.asarray(w_kv, np.float32)
    w_o = np.asarray(w_o, np.float32)
    cos_t, sin_t = _rope_tabs()
    qsc = np.ascontiguousarray(
        np.asarray(q_norm_scale, np.float32).reshape(HC, 128).T)
    ksc = np.ascontiguousarray(
        np.asarray(k_norm_scale, np.float32).reshape(HC, 128).T)

    p = np.arange(128)[:, None]
    f = np.arange(128)[None, :]
    mdiag = np.where(p >= f, 0.0, NEG).astype(np.float32)
    medge = np.where(f >= p + 1, 0.0, NEG).astype(np.float32)

    xT_b = []
    for b in range(B):
        xT_b.append(_tile128(np.ascontiguousarray(x[b].T).astype(BF16)))

    in_maps = []
    for c in range(8):
        b, kp = c // 4, c % 4
        n0, k0 = 4 * kp, 2 * kp
        wq = _tile128(w_q[n0:n0 + 4].transpose(1, 0, 2).reshape(D, QH * H).astype(BF16))
        wk = w_kv[0, k0:k0 + 2].transpose(1, 0, 2).reshape(D, KVH * H)
        wv = w_kv[1, k0:k0 + 2].transpose(1, 0, 2).reshape(D, KVH * H)
        wkv = _tile128(np.concatenate([wk, wv], axis=1).astype(BF16))
        wo = _tile128(w_o[n0:n0 + 4].reshape(QH * H, D).astype(BF16))
        m = {"xT": xT_b[b], "wq": wq, "wkv": wkv, "wo": wo,
             "mdiag": mdiag, "medge": medge,
             "cos": cos_t, "sin": sin_t, "qsc": qsc, "ksc": ksc}
        in_maps.append(m)

    res = run_bass_kernel_spmd(nc, in_maps, core_ids=list(range(8)))
    _cached["last_result"] = res
    y = np.zeros((B, T, D), np.float32)
    for c in range(8):
        y[c // 4] += np.asarray(res.results[c]["out"], np.float32)
    return y


# revision 19
# speedup vs baseline: 1.0174x; 1.0174x over previous
"""Sliding-window GQA attention on 8 TRN2 NeuronCores.

Sharding: core c handles batch b=c//4 and kv-head pair 2*(c%4)..+1
(-> 4 query heads, 2 kv heads, all 2048 tokens of one batch).
Each core computes its heads' partial o-projection [2048, 3584];
the host sums the 4 partials per batch. No on-device collectives.

All heavy matmuls run in bf16 (fp32 PSUM accumulate). Host pre-lays-out
x^T / weights in [128-partition, ...] tiles so every DMA is contiguous.
RoPE cos/sin tables and the two 128x128 additive mask tiles (causal
diag, window edge) come from host; the QK-norm scale is applied
post-transpose where the head dim sits on partitions.
"""

import os
import numpy as np
import ml_dtypes

B, T, D, H = 2, 2048, 3584, 256
QH, KVH = 4, 2          # per-core q heads / kv heads
DC = D // 128           # 28 contract chunks
TBN = T // 128          # 16 token blocks
HC = H // 128           # 2 head-dim chunks
OC = QH * H // 128      # 8 out-proj contract chunks
SCALE = 0.0625
EPS = 1e-6
ROPE_BASE = 10000.0
WB = 1024 // 128        # window in blocks (8)
NEG = -1.0e30
NDOUT = D // 512        # 7 o-proj column chunks

BF16 = ml_dtypes.bfloat16

_cached = {}


def _build():
    import concourse.bass as bass
    import concourse.mybir as mybir
    import concourse.tile as tile
    from concourse import bacc
    from concourse.masks import make_identity

    f32 = mybir.dt.float32
    bf16 = mybir.dt.bfloat16
    AF = mybir.ActivationFunctionType

    nc = bacc.Bacc(None, target_bir_lowering=False)

    xT_d = nc.dram_tensor("xT", [128, DC, T], bf16, kind="ExternalInput")
    wq_d = nc.dram_tensor("wq", [128, DC, QH * H], bf16, kind="ExternalInput")
    wkv_d = nc.dram_tensor("wkv", [128, DC, 2 * KVH * H], bf16, kind="ExternalInput")
    wo_d = nc.dram_tensor("wo", [128, OC, D], bf16, kind="ExternalInput")
    cos_d = nc.dram_tensor("cos", [128, TBN, 128], f32, kind="ExternalInput")
    sin_d = nc.dram_tensor("sin", [128, TBN, 128], f32, kind="ExternalInput")
    qsc_d = nc.dram_tensor("qsc", [128, HC], f32, kind="ExternalInput")
    ksc_d = nc.dram_tensor("ksc", [128, HC], f32, kind="ExternalInput")
    mdiag_d = nc.dram_tensor("mdiag", [128, 128], f32, kind="ExternalInput")
    medge_d = nc.dram_tensor("medge", [128, 128], f32, kind="ExternalInput")
    out_d = nc.dram_tensor("out", [T, D], f32, kind="ExternalOutput")

    DCQ = DC // 4  # 7 d-chunks per weight quarter

    with tile.TileContext(nc) as tc:
        with (
            tc.tile_pool(name="persist", bufs=1) as pers,
            tc.tile_pool(name="wpool", bufs=6) as wpool,
        ):
            qT = pers.tile([128, HC, QH, T], bf16)     # q^T  [h, hc, head, t]
            kT = pers.tile([128, HC, KVH, T], bf16)    # k^T  [h, hc, kv, s]
            vS = pers.tile([128, TBN, KVH, H], bf16)   # v    [s, sblock, kv, h]
            ident = pers.tile([128, 128], bf16)
            make_identity(nc, ident)
            epsb = pers.tile([128, 1], f32)
            nc.gpsimd.memset(epsb, EPS)
            epsb2 = pers.tile([128, 1], f32)
            nc.gpsimd.memset(epsb2, EPS / (SCALE * SCALE))
            rqs = pers.tile([128, TBN, QH], f32)   # SCALE/rms(q) per (t, head)

            def load_w_quarters(dram, rows, cols, eng=None):
                parts = []
                for qi in range(4):
                    wt = wpool.tile([128, rows // 4, cols], bf16, tag="w",
                                    name=f"w_{qi}")
                    (eng or nc.sync).dma_start(
                        wt, dram[:, qi * (rows // 4):(qi + 1) * (rows // 4), :])
                    parts.append(wt)
                return parts

            # ---------------- phase 1: projections ----------------
            with (
                tc.tile_pool(name="tabs", bufs=1) as ptab,
                tc.tile_pool(name="xt", bufs=3) as pxt,
                tc.tile_pool(name="scr", bufs=3) as scr,
                tc.tile_pool(name="ppq", bufs=6, space=bass.MemorySpace.PSUM) as ppq,
                tc.tile_pool(name="ptr", bufs=2, space=bass.MemorySpace.PSUM) as ptrp,
            ):
                # wq quarter 0 gets the sync queue to itself; quarters 1-3
                # are issued on the paced gpsimd queue between early blocks.
                WSPLIT = (4, 8, 8, 8)
                WOFF = (0, 4, 12, 20)
                wq_p = [wpool.tile([128, WSPLIT[qi], QH * H], bf16, tag="w",
                                   name=f"wq_{qi}") for qi in range(4)]
                nc.sync.dma_start(wq_p[0], wq_d[:, :4, :])

                xts0 = []
                for tbe in range(3):
                    xte = pxt.tile([128, DC, 128], bf16, tag="xt", name="xt0")
                    nc.gpsimd.dma_start(xte, xT_d[:, :, tbe * 128:(tbe + 1) * 128])
                    xts0.append(xte)
                for qi in range(1, 4):
                    nc.gpsimd.dma_start(
                        wq_p[qi], wq_d[:, WOFF[qi]:WOFF[qi] + WSPLIT[qi], :])

                cost = ptab.tile([128, TBN, 128], f32)
                sint = ptab.tile([128, TBN, 128], f32)
                qsc = ptab.tile([128, HC], f32)
                ksc = ptab.tile([128, HC], f32)
                nc.gpsimd.dma_start(cost, cos_d[:])
                nc.gpsimd.dma_start(sint, sin_d[:])
                nc.gpsimd.dma_start(qsc, qsc_d[:])
                nc.gpsimd.dma_start(ksc, ksc_d[:])

                def proj_epilogue(pq, j, tb, scv, dstT, slot, defer_rstd=False):
                    """norm+rope head j of psum pq -> transpose into dstT[:, hc, slot, tb].
                    The QK-norm per-h scale is applied post-transpose (h on
                    partitions). For q (defer_rstd), 1/rms is not applied here;
                    SCALE/rms(q) is stored in rqs and folded into the exp scale."""
                    sq = scr.tile([128, H], f32, tag="sq")
                    ssq = scr.tile([128, 1], f32, tag="ssq")
                    nc.scalar.activation(sq, pq[:, j, :], AF.Square, accum_out=ssq)
                    std = scr.tile([128, 1], f32, tag="std")
                    if defer_rstd:
                        nc.scalar.activation(std, ssq, AF.Sqrt, bias=epsb2[:, 0:1],
                                             scale=1.0 / (H * SCALE * SCALE))
                        nc.vector.reciprocal(rqs[:, tb, slot, None], std)
                    else:
                        nc.scalar.activation(std, ssq, AF.Sqrt, bias=epsb[:, 0:1],
                                             scale=1.0 / H)
                        rstd = scr.tile([128, 1], f32, tag="rstd")
                        nc.vector.reciprocal(rstd, std)
                        rb = rstd[:, 0:1].to_broadcast((128, 128))
                    x1 = pq[:, j, 0:128]
                    x2 = pq[:, j, 128:256]
                    t1 = scr.tile([128, 128], f32, tag="t1")
                    qr = scr.tile([128, H], bf16, tag="qr")
                    t2 = scr.tile([128, 128], f32, tag="t2")
                    nc.vector.tensor_mul(t1, x1, cost[:, tb, :])
                    nc.vector.tensor_mul(t2, x2, sint[:, tb, :])
                    if defer_rstd:
                        nc.vector.tensor_sub(qr[:, 0:128], t1, t2)
                    else:
                        nc.vector.tensor_sub(t1, t1, t2)
                        nc.vector.tensor_mul(qr[:, 0:128], t1, rb)
                    nc.vector.tensor_mul(t1, x2, cost[:, tb, :])
                    nc.vector.tensor_mul(t2, x1, sint[:, tb, :])
                    if defer_rstd:
                        nc.vector.tensor_add(qr[:, 128:256], t1, t2)
                    else:
                        nc.vector.tensor_add(t1, t1, t2)
                        nc.vector.tensor_mul(qr[:, 128:256], t1, rb)
                    for hc in range(HC):
                        ptr = ptrp.tile([128, 128], bf16, tag="ptr")
                        nc.tensor.transpose(ptr, qr[:, hc * 128:(hc + 1) * 128], ident)
                        nc.vector.tensor_mul(
                            dstT[:, hc, slot, tb * 128:(tb + 1) * 128], ptr,
                            scv[:, hc:hc + 1].to_broadcast((128, 128)))

                def proj_block(tb, w_p, nhead, xt=None, woff=None):
                    if xt is None:
                        xt = pxt.tile([128, DC, 128], bf16, tag="xt")
                        nc.gpsimd.dma_start(xt, xT_d[:, :, tb * 128:(tb + 1) * 128])
                    def wsel(dc):
                        if woff is None:
                            return w_p[dc // DCQ], dc % DCQ
                        for qi in range(3, -1, -1):
                            if dc >= woff[qi]:
                                return w_p[qi], dc - woff[qi]
                    pqa = ppq.tile([128, 2, H], f32, tag="pq", name="pqa")
                    pqb = ppq.tile([128, 2, H], f32, tag="pq", name="pqb")
                    for dc in range(DC):
                        lhsT = xt[:, dc, :]
                        wt, dcl = wsel(dc)
                        nc.tensor.matmul(pqa[:, :, :], lhsT, wt[:, dcl, 0:512],
                                         start=(dc == 0), stop=(dc == DC - 1))
                        nc.tensor.matmul(pqb[:, :, :], lhsT, wt[:, dcl, 512:1024],
                                         start=(dc == 0), stop=(dc == DC - 1))
                    return (pqa, pqb)

                # --- 1a: Q ---
                wkv_p = None
                for tb in range(TBN):
                    pq2 = proj_block(tb, wq_p, QH,
                                     xt=xts0[tb] if tb < 3 else None,
                                     woff=WOFF)
                    for j in range(QH):
                        proj_epilogue(pq2[j // 2], j % 2, tb, qsc, qT, j,
                                      defer_rstd=True)
                    if tb == 7:
                        # paced prefetch: issued on the xt DMA queue mid-phase
                        wkv_p = load_w_quarters(wkv_d, DC, 2 * KVH * H,
                                                eng=nc.gpsimd)

                # --- 1b: K and V ---
                for tb in range(TBN):
                    pq2 = proj_block(tb, wkv_p, 2 * KVH)
                    for kv in range(KVH):
                        proj_epilogue(pq2[0], kv, tb, ksc, kT, kv)
                    for kv in range(KVH):
                        nc.vector.tensor_copy(vS[:, tb, kv, :], pq2[1][:, kv, :])
                    if tb == 7:
                        wo_p = []
                        for qi in range(4):
                            wt = wpool.tile([128, OC // 4, D], bf16, tag="w",
                                            name=f"wo_{qi}")
                            nc.gpsimd.dma_start(
                                wt, wo_d[:, qi * (OC // 4):(qi + 1) * (OC // 4), :])
                            wo_p.append(wt)

            # ---------------- phase 2: attention + o-proj ----------------
            with (
                tc.tile_pool(name="att", bufs=2) as att,
                tc.tile_pool(name="ysb", bufs=1) as pys,
                tc.tile_pool(name="pl", bufs=2, space=bass.MemorySpace.PSUM) as plp,
                tc.tile_pool(name="pt2", bufs=3, space=bass.MemorySpace.PSUM) as pt2,
                tc.tile_pool(name="po", bufs=1, space=bass.MemorySpace.PSUM) as pop,
                tc.tile_pool(name="py", bufs=2, space=bass.MemorySpace.PSUM) as pyp,
            ):
                mdiag = att.tile([128, 128], f32, tag="mdiag", bufs=1)
                medge = att.tile([128, 128], f32, tag="medge", bufs=1)
                nc.sync.dma_start(mdiag, mdiag_d[:])
                nc.sync.dma_start(medge, medge_d[:])


                for tb in range(TBN):
                    sb0 = max(0, tb - WB)
                    ns = tb - sb0 + 1
                    sw = ns * 128
                    dg = (tb - sb0) * 128  # diag block offset within row
                    outsb = att.tile([128, QH, H], bf16, tag="outsb")
                    for j in range(QH):
                        kv = j // 2
                        # QK in 512-wide chunks; per-chunk exp so PE/ACT pipeline
                        pcs = []
                        sumes = []
                        for c0 in range(0, sw, 512):
                            cols = min(512, sw - c0)
                            pl = plp.tile([128, 512], f32, tag="pl", name="pl")
                            for hc in range(HC):
                                nc.tensor.matmul(
                                    pl[:, :cols],
                                    qT[:, hc, j, tb * 128:(tb + 1) * 128],
                                    kT[:, hc, kv,
                                       sb0 * 128 + c0: sb0 * 128 + c0 + cols],
                                    start=(hc == 0), stop=(hc == HC - 1))
                            if tb >= WB and c0 == 0:
                                nc.vector.tensor_add(pl[:, 0:128], pl[:, 0:128], medge)
                            if c0 <= dg < c0 + cols:
                                off = dg - c0
                                nc.vector.tensor_add(pl[:, off:off + 128],
                                                     pl[:, off:off + 128], mdiag)
                            pc = att.tile([128, 512], bf16, tag="probs", bufs=6,
                                          name="pc")
                            se = att.tile([128, 1], f32, tag="sume", bufs=8, name="se")
                            nc.scalar.activation(pc[:, :cols], pl[:, :cols], AF.Exp,
                                                 scale=rqs[:, tb, j, None],
                                                 accum_out=se)
                            pcs.append(pc)
                            sumes.append(se)
                        tot = sumes[0]
                        for se in sumes[1:]:
                            t2 = att.tile([128, 1], f32, tag="sume", bufs=8, name="se2")
                            nc.vector.tensor_add(t2, tot, se)
                            tot = t2
                        recip = att.tile([128, 1], f32, tag="recip")
                        nc.vector.reciprocal(recip, tot)
                        pT = att.tile([128, 9, 128], bf16, tag="pT")
                        for s in range(ns):
                            ptr = pt2.tile([128, 128], bf16, tag="ptr2")
                            nc.tensor.transpose(
                                ptr, pcs[s // 4][:, (s % 4) * 128:(s % 4 + 1) * 128],
                                ident)
                            if s % 3 != 2:
                                nc.vector.tensor_copy(pT[:, s, :], ptr)
                            else:
                                nc.scalar.activation(pT[:, s, :], ptr, AF.Copy)
                        po = pop.tile([128, H], f32, tag="po")
                        for s in range(ns):
                            nc.tensor.matmul(po, pT[:, s, :], vS[:, sb0 + s, kv, :],
                                             start=(s == 0), stop=(s == ns - 1))
                        nc.vector.tensor_mul(outsb[:, j, :], po,
                                             recip[:, 0:1].to_broadcast((128, H)))
                    outT = att.tile([128, OC, 128], bf16, tag="outT")
                    for c in range(OC):
                        ptr = pt2.tile([128, 128], bf16, tag="ptr2")
                        nc.tensor.transpose(
                            ptr, outsb[:, c // 2, (c % 2) * 128:(c % 2 + 1) * 128],
                            ident)
                        if c % 2 == 0:
                            nc.vector.tensor_copy(outT[:, c, :], ptr)
                        else:
                            nc.scalar.activation(outT[:, c, :], ptr, AF.Copy)
                    ysb = pys.tile([128, D], f32, tag="y")
                    for dx in range(NDOUT):
                        py = pyp.tile([128, 512], f32, tag="py")
                        for c in range(OC):
                            nc.tensor.matmul(py, outT[:, c, :],
                                             wo_p[c // 2][:, c % 2,
                                                          dx * 512:(dx + 1) * 512],
                                             start=(c == 0), stop=(c == OC - 1))
                        if dx % 2 == 0:
                            nc.vector.tensor_copy(ysb[:, dx * 512:(dx + 1) * 512], py)
                        else:
                            nc.scalar.activation(ysb[:, dx * 512:(dx + 1) * 512], py,
                                                 AF.Copy)
                    nc.sync.dma_start(out_d[tb * 128:(tb + 1) * 128, :D // 2],
                                      ysb[:, :D // 2])
                    nc.sync.dma_start(out_d[tb * 128:(tb + 1) * 128, D // 2:],
                                      ysb[:, D // 2:])

    nc.compile()
    return nc


def _tile128(a):
    """[128*n, m] -> [128, n, m] with row index = chunk*128 + partition."""
    n = a.shape[0] // 128
    return np.ascontiguousarray(
        a.reshape(n, 128, *a.shape[1:]).transpose(1, 0, *range(2, a.ndim + 1)))


def _rope_tabs():
    j = np.arange(128, dtype=np.float64)
    ts = ROPE_BASE ** (2.0 * j / H)
    ang = np.arange(T, dtype=np.float64)[:, None] / ts[None, :]
    return (_tile128(np.cos(ang).astype(np.float32)),
            _tile128(np.sin(ang).astype(np.float32)))


def kernel(x, w_q, w_kv, w_o, q_norm_scale, k_norm_scale):
    from concourse.bass_utils import run_bass_kernel_spmd

    if "nc" not in _cached:
        _cached["nc"] = _build()
    nc = _cached["nc"]

    x = np.asarray(x, np.float32)
    w_q = np.asarray(w_q, np.float32)
    w_kv = np.asarray(w_kv, np.float32)
    w_o = np.asarray(w_o, np.float32)
    cos_t, sin_t = _rope_tabs()
    qsc = np.ascontiguousarray(
        np.asarray(q_norm_scale, np.float32).reshape(HC, 128).T)
    ksc = np.ascontiguousarray(
        np.asarray(k_norm_scale, np.float32).reshape(HC, 128).T)

    p = np.arange(128)[:, None]
    f = np.arange(128)[None, :]
    mdiag = np.where(p >= f, 0.0, NEG).astype(np.float32)
    medge = np.where(f >= p + 1, 0.0, NEG).astype(np.float32)

    xT_b = []
    for b in range(B):
        xT_b.append(_tile128(np.ascontiguousarray(x[b].T).astype(BF16)))

    in_maps = []
    for c in range(8):
        b, kp = c // 4, c % 4
        n0, k0 = 4 * kp, 2 * kp
        wq = _tile128(w_q[n0:n0 + 4].transpose(1, 0, 2).reshape(D, QH * H).astype(BF16))
        wk = w_kv[0, k0:k0 + 2].transpose(1, 0, 2).reshape(D, KVH * H)
        wv = w_kv[1, k0:k0 + 2].transpose(1, 0, 2).reshape(D, KVH * H)
        wkv = _tile128(np.concatenate([wk, wv], axis=1).astype(BF16))
        wo = _tile128(w_o[n0:n0 + 4].reshape(QH * H, D).astype(BF16))
        m = {"xT": xT_b[b], "wq": wq, "wkv": wkv, "wo": wo,
             "mdiag": mdiag, "medge": medge,
             "cos": cos_t, "sin": sin_t, "qsc": qsc, "ksc": ksc}
        in_maps.append(m)

    res = run_bass_kernel_spmd(nc, in_maps, core_ids=list(range(8)))
    _cached["last_result"] = res
    y = np.zeros((B, T, D), np.float32)
    for c in range(8):
        y[c // 4] += np.asarray(res.results[c]["out"], np.float32)
    return y
